# revision 22
# baseline (speedup 1.0000x reference)
"""EdgeConv (DGCNN-style) Bass kernel for 8 Trainium2 NeuronCores — v3.

Math (reference):
    local = W1 @ feature              (B, 64, N)
    edge  = W2 @ feature              (B, 64, N)
    nbr[b,c,n,j] = edge[b,c,idx[b,n,j]]
    ef = concat([central bcast, nbr - central], ch) -> BN(eval) -> relu
    out = mean over j                 (B, 128, N)

BN folded (eval mode):
    inv = gamma / sqrt(var + eps); shift = beta - mean * inv
    out1[c,n] = relu((inv1*W1) @ feat + shift1)                  -- no gather
    out2[c,n] = (1/K) * sum_j max(e[c,idx], u[c,n]) - u[c,n]
      with e = (inv2*W2) @ feat  (gather table), u = (inv2*W1) @ feat - shift2

Perf notes (measured this session): the KNN gather is the wall. SWDGE
indirect gathers are packet-rate-bound at ~2.5ns/256B-packet aggregate
with all 4 SWDGE queues saturated (~100GB/s); source (HBM vs SBUF) and
elem size barely matter, single_packet=True and 128B elems crash the
ucode, and transpose-mode output is corrupted when gathers run on >1
queue concurrently.  So: HBM table [N, 128] bf16 (64 e-channels + 64
zero pad -> 256B rows), non-transpose gathers round-robin on 4 queues
with 7 gather buffers so all queues stay busy, and everything else
(table build, u, out1, DVE max + K-sum tree, output writes) overlaps
under the gather.  Points-on-partitions gather output keeps the DVE
ops 128 partitions wide.

Sharding: core = 2*b + h handles batch b, half h of the N points.
Host feat4 puts the core's own half-quarters in blocks 0,1 so the SPMD
program is core-agnostic; the table slot remap accounts for it.
"""

import os
import sys

for _p in ("/opt/trn_rl_repo",):
    if _p not in sys.path:
        sys.path.insert(0, _p)

import numpy as np

import concourse.bass as bass
import concourse.bacc as bacc
import concourse.mybir as mybir
import concourse.tile as tile

F32 = mybir.dt.float32
BF16 = mybir.dt.bfloat16
I16 = mybir.dt.int16

BN_EPS = 1e-5


def _to_bf16(a):
    import ml_dtypes
    return np.asarray(a, np.float32).astype(ml_dtypes.bfloat16)


GQ = int(os.environ.get("EDGECONV_QUEUES", "4"))      # SWDGE queues
GBUFS = int(os.environ.get("EDGECONV_BUFS", "7"))     # gather tile buffers


def full_cfg():
    return dict(B=4, CIN=32, C=64, N=32768, K=16)


def derived(cfg):
    d = dict(cfg)
    d["Q"] = cfg["N"] // 4          # tokens per quarter
    d["NP"] = cfg["N"] // 2         # points per core
    d["P"] = 512                    # points per gather chunk
    d["SUB"] = d["P"] // 128        # point sub-tiles per chunk (=4)
    d["NCH"] = d["NP"] // d["P"]    # chunks per core
    d["NI"] = d["P"] * cfg["K"]     # idxs per chunk (8192)
    d["NT"] = d["Q"] // 128         # table matmul tiles
    return d


def build_bass(cfg):
    """Build the single-core SPMD program. Returns finalized Bass."""
    d = derived(cfg)
    CIN, C, N, K, Q = d["CIN"], d["C"], d["N"], d["K"], d["Q"]
    NP, P, SUB, NCH, NI, NT = d["NP"], d["P"], d["SUB"], d["NCH"], d["NI"], d["NT"]

    nc = bacc.Bacc("TRN2", target_bir_lowering=False, debug=False,
                   num_swdge_queues=GQ)

    # ---- I/O ----
    feat4 = nc.dram_tensor("feat4", [4 * CIN, Q], BF16, kind="ExternalInput").ap()
    wc_pad = nc.dram_tensor("wc_pad", [4 * CIN, 4 * 128], BF16, kind="ExternalInput").ap()
    wb_blk = nc.dram_tensor("wb_blk", [2 * CIN, 2 * C], BF16, kind="ExternalInput").ap()
    waT_rep = nc.dram_tensor("waT_rep", [2 * CIN, C], BF16, kind="ExternalInput").ap()
    s1_d = nc.dram_tensor("s1", [C, 1], F32, kind="ExternalInput").ap()
    sh2_d = nc.dram_tensor("sh2_rep", [128, 2 * C], F32, kind="ExternalInput").ap()
    idx_d = nc.dram_tensor("idx", [NCH // 4, 128, 4, NI // 16], I16, kind="ExternalInput").ap()
    out1 = nc.dram_tensor("out1", [C, NP], BF16, kind="ExternalOutput").ap()
    out2 = nc.dram_tensor("out2", [NCH // 4, 128, 4, SUB, C], BF16, kind="ExternalOutput").ap()
    table = nc.dram_tensor("table", [N, 128], BF16, kind="Internal").ap()
    # token slot r = (n_blocked % Q)*4 + block; nested (m, p, q): grouped
    # writes of 4 matmul tiles -> 4KB contiguous per partition line
    tab_g = table.rearrange("(m p q) c -> p m (q c)", p=128, q=4)

    inv_k = 1.0 / K

    with tile.TileContext(nc) as tc:
        with (
            tc.tile_pool(name="persist", bufs=1) as pp,
            tc.tile_pool(name="tabw", bufs=2) as tw,
            tc.tile_pool(name="idxp", bufs=2) as ip,
            tc.tile_pool(name="gath", bufs=GBUFS) as gp,
            tc.tile_pool(name="tree", bufs=1) as tp,
            tc.tile_pool(name="outp", bufs=2) as wp,
            tc.tile_pool(name="psum", bufs=2, space="PSUM") as pm,
        ):
            # ---- persistent SBUF ----
            feat4_sb = pp.tile([4 * CIN, Q], BF16)
            wc_sb = pp.tile([4 * CIN, 4 * 128], BF16)
            wb_sb = pp.tile([2 * CIN, 2 * C], BF16)
            wa_sb = pp.tile([2 * CIN, C], BF16)
            s1_sb = pp.tile([C, 1], F32)
            sh2_sb = pp.tile([128, 2 * C], F32)
            u_sb = pp.tile([128, NP // 128, C], BF16)

            nc.sync.dma_start(out=feat4_sb[:], in_=feat4[:])
            nc.sync.dma_start(out=wc_sb[:], in_=wc_pad[:])
            nc.sync.dma_start(out=wb_sb[:], in_=wb_blk[:])
            nc.sync.dma_start(out=wa_sb[:], in_=waT_rep[:])
            nc.sync.dma_start(out=s1_sb[:], in_=s1_d[:])
            nc.sync.dma_start(out=sh2_sb[:], in_=sh2_d[:])

            # ---- phase T: build gather table in HBM ----
            # wc_pad block q: rows q*CIN..(q+1)CIN, cols q*128..q*128+64 = Wc^T
            # (cols +64..+128 zero -> zero pad channels, no memset needed)
            tb4 = None
            for m in range(NT):
                ps = pm.tile([128, 4 * 128], F32, tag="tab")
                nc.tensor.matmul(
                    out=ps[:],
                    lhsT=feat4_sb[:, m * 128 : (m + 1) * 128],
                    rhs=wc_sb[:],
                    start=True,
                    stop=True,
                )
                if m % 4 == 0:
                    tb4 = tw.tile([128, 4, 512], BF16, tag="tb")
                nc.scalar.copy(out=tb4[:, m % 4, :], in_=ps[:])
                if m % 4 == 3:
                    nc.sync.dma_start(
                        out=tab_g[:, m - 3 : m + 1, :], in_=tb4[:])

            # ---- phase U: u = (inv2*W1) @ feat_own_half - shift2 ----
            # feat4 blocks 0,1 = the core's own half (host-reordered).
            # Block-diag wb over the 2 own quarters: ps[p, u*64+c] =
            # u'[c, token u*Q + m*128 + p]
            u_v = u_sb[:].rearrange("p (u q) c -> p u q c", u=2)
            for it in range(NT):
                m0 = it * 128
                ps = pm.tile([128, 2 * C], F32, tag="u")
                nc.tensor.matmul(
                    out=ps[:],
                    lhsT=feat4_sb[0 : 2 * CIN, m0 : m0 + 128],
                    rhs=wb_sb[:],
                    start=True,
                    stop=True,
                )
                nc.vector.scalar_tensor_tensor(
                    out=u_v[:, :, it, :],
                    in0=ps[:].rearrange("p (u c) -> p u c", c=C),
                    scalar=1.0,
                    in1=sh2_sb[:].rearrange("p (u c) -> p u c", c=C),
                    op0=mybir.AluOpType.mult,
                    op1=mybir.AluOpType.subtract,
                )

            # ---- phase G: gather + max + K-tree + fixup; out1 interleaved ----
            o1_tiles = [(u, m0) for u in range(2) for m0 in range(0, Q, P)]
            sw_i = 0  # global SWDGE-DMA instr counter: DMASW sem lanes are
            # handed out round-robin per instruction and lock to the first
            # queue that uses them, so queue_num must follow sw_i % GQ
            idx4 = None
            o24 = None
            o14 = None
            for g in range(NCH):
                if g % 4 == 0:
                    idx4 = ip.tile([128, 4, NI // 16], I16, tag="idx")
                    nc.sync.dma_start(out=idx4[:], in_=idx_d[g // 4])
                idx_sb = idx4[:, g % 4]
                gt = gp.tile([128, NI // 128, 128], BF16, tag="g")
                # first instruction on each queue serializes gen->drain;
                # split it in two so the queue starts draining sooner
                nsplit = 4 if g < GQ else 1
                ni_s = NI // nsplit
                for sp in range(nsplit):
                    nc.gpsimd.dma_gather(
                        out_ap=gt[:, sp * (ni_s // 128) : (sp + 1) * (ni_s // 128), :],
                        in_ap=table[:],
                        idxs_ap=idx_sb[:, sp * (ni_s // 16) : (sp + 1) * (ni_s // 16)],
                        num_idxs=ni_s,
                        num_idxs_reg=ni_s,
                        elem_size=128,
                        transpose=False,
                        single_packet=False,
                        queue_num=sw_i % GQ,
                    )
                    sw_i += 1
                # slot (p, a*K+j) holds point n = g*512 + a*128 + p, nbr j
                gt_v = gt[:].rearrange("p (a k) c -> p a k c", k=K)[:, :, :, 0:C]
                u_g = u_sb[:, SUB * g : SUB * (g + 1), :]
                m1 = tp.tile([128, SUB, K, C], BF16, tag="m1")
                nc.vector.tensor_tensor(
                    out=m1[:],
                    in0=gt_v,
                    in1=u_g[:, :, None, :].broadcast_to((128, SUB, K, C)),
                    op=mybir.AluOpType.max,
                )
                t8 = tp.tile([128, SUB, K // 2, C], BF16, tag="t8")
                nc.vector.tensor_add(
                    out=t8[:], in0=m1[:, :, 0:8, :], in1=m1[:, :, 8:16, :]
                )
                nc.vector.tensor_add(
                    out=m1[:, :, 0:4, :], in0=t8[:, :, 0:4, :], in1=t8[:, :, 4:8, :]
                )
                nc.vector.tensor_add(
                    out=t8[:, :, 0:2, :], in0=m1[:, :, 0:2, :], in1=m1[:, :, 2:4, :]
                )
                s = tp.tile([128, SUB, 1, C], F32, tag="ts")
                nc.vector.tensor_add(
                    out=s[:], in0=t8[:, :, 0:1, :], in1=t8[:, :, 1:2, :]
                )
                if g % 4 == 0:
                    o24 = wp.tile([128, 4, SUB, C], BF16, tag="o2")
                nc.vector.scalar_tensor_tensor(
                    out=o24[:, g % 4],
                    in0=s[:, :, 0, :],
                    scalar=inv_k,
                    in1=u_g[:],
                    op0=mybir.AluOpType.mult,
                    op1=mybir.AluOpType.subtract,
                )
                if g % 4 == 3:
                    nc.sync.dma_start(out=out2[g // 4], in_=o24[:])

                if g < len(o1_tiles):
                    u, m0 = o1_tiles[g]
                    ps = pm.tile([C, P], F32, tag="o1")
                    nc.tensor.matmul(
                        out=ps[:],
                        lhsT=wa_sb[u * CIN : (u + 1) * CIN, :],
                        rhs=feat4_sb[u * CIN : (u + 1) * CIN, m0 : m0 + P],
                        start=True,
                        stop=True,
                    )
                    if g % 4 == 0:
                        o14 = wp.tile([C, 4, P], BF16, tag="o1sb")
                    nc.scalar.activation(
                        out=o14[:, g % 4],
                        in_=ps[:],
                        func=mybir.ActivationFunctionType.Relu,
                        bias=s1_sb[:],
                        scale=1.0,
                    )
                    if g % 4 == 3:
                        nc.sync.dma_start(
                            out=out1[:, (g - 3) * P : (g + 1) * P], in_=o14[:]
                        )

    nc.compile()
    return nc


def host_prep(cfg, feature, knn_inds, W1, W2, bn_gamma, bn_beta, bn_mean, bn_var):
    """Fold BN into weights, shard + lay out per-core inputs (numpy only)."""
    d = derived(cfg)
    B, CIN, C, N, K, Q = d["B"], d["CIN"], d["C"], d["N"], d["K"], d["Q"]
    NP, P, NCH, NI, SUB = d["NP"], d["P"], d["NCH"], d["NI"], d["SUB"]

    feature = np.asarray(feature, np.float32)
    knn = np.asarray(knn_inds)
    inv = (np.asarray(bn_gamma, np.float32)
           / np.sqrt(np.asarray(bn_var, np.float32) + BN_EPS))
    shift = np.asarray(bn_beta, np.float32) - np.asarray(bn_mean, np.float32) * inv
    inv1, inv2 = inv[:C], inv[C:]
    s1, sh2 = shift[:C], shift[C:]
    Wa = (inv1[:, None] * np.asarray(W1, np.float32)).astype(np.float32)
    Wb = (inv2[:, None] * np.asarray(W1, np.float32)).astype(np.float32)
    Wc = (inv2[:, None] * np.asarray(W2, np.float32)).astype(np.float32)

    wc_pad = np.zeros((4 * CIN, 4 * 128), np.float32)
    for q in range(4):
        wc_pad[q * CIN : (q + 1) * CIN, q * 128 : q * 128 + C] = Wc.T
    wb_blk = np.zeros((2 * CIN, 2 * C), np.float32)
    for u in range(2):
        wb_blk[u * CIN : (u + 1) * CIN, u * C : (u + 1) * C] = Wb.T
    waT_rep = np.tile(Wa.T, (2, 1))
    wc_pad, wb_blk, waT_rep = map(_to_bf16, (wc_pad, wb_blk, waT_rep))
    s1_col = np.ascontiguousarray(s1.reshape(C, 1))
    sh2_rep = np.ascontiguousarray(
        np.broadcast_to(np.tile(sh2, 2), (128, 2 * C)), dtype=np.float32)

    in_maps = []
    for core in range(8):
        b, h = core // 2, core % 2
        f = feature[b].reshape(CIN, 4, Q)
        qorder = [2 * h, 2 * h + 1, 2 * (1 - h), 2 * (1 - h) + 1]
        feat4 = _to_bf16(np.ascontiguousarray(
            f[:, qorder].transpose(1, 0, 2).reshape(4 * CIN, Q)))
        binv = np.empty(4, np.int64)
        for bq, q in enumerate(qorder):
            binv[q] = bq
        kn = knn[b, h * NP : (h + 1) * NP].astype(np.int64)   # (NP, K)
        # table slot: r = (n % Q)*4 + block(n)
        r = (kn % Q) * 4 + binv[kn // Q]
        # stream order: i = (a*K + j)*128 + p for point g*512 + a*128 + p
        st = (r.reshape(NCH, SUB, 128, K).transpose(0, 1, 3, 2)
              .reshape(NCH, NI))
        wrap = st.reshape(NCH, NI // 16, 16).transpose(0, 2, 1)
        ridx = (np.broadcast_to(wrap[:, None, :, :], (NCH, 8, 16, NI // 16))
                .reshape(NCH // 4, 4, 128, NI // 16)
                .transpose(0, 2, 1, 3).astype(np.int16))
        in_maps.append({
            "feat4": feat4, "wc_pad": wc_pad, "wb_blk": wb_blk,
            "waT_rep": waT_rep, "s1": s1_col, "sh2_rep": sh2_rep,
            "idx": np.ascontiguousarray(ridx),
        })
    return in_maps


def extract_core(cfg, core, res):
    """Return (out1, out2) as [C, NP] for one core's results."""
    d = derived(cfg)
    C, NP = d["C"], d["NP"]
    o2 = np.asarray(res["out2"], np.float32)
    o2 = o2.transpose(0, 2, 3, 1, 4).reshape(NP, C).T
    return np.asarray(res["out1"], np.float32), o2


def assemble(cfg, results):
    """Reassemble the full (B, 2C, N) output from 8 per-core results."""
    d = derived(cfg)
    B, C, N, NP = d["B"], d["C"], d["N"], d["NP"]
    out = np.empty((B, 2 * C, N), np.float32)
    for core in range(8):
        b, h = core // 2, core % 2
        o1, o2 = extract_core(cfg, core, results[core])
        sl = slice(h * NP, (h + 1) * NP)
        out[b, :C, sl] = o1
        out[b, C:, sl] = o2
    return out


_CACHED = {}


def _get_nc(cfg_key):
    if cfg_key not in _CACHED:
        _CACHED[cfg_key] = build_bass(full_cfg())
    return _CACHED[cfg_key]


def kernel(feature, knn_inds, W1, W2, bn_gamma, bn_beta, bn_mean, bn_var):
    from concourse.bass_utils import run_bass_kernel_spmd

    cfg = full_cfg()
    nc = _get_nc("full")
    in_maps = host_prep(cfg, feature, knn_inds, W1, W2,
                        bn_gamma, bn_beta, bn_mean, bn_var)
    trace = bool(int(os.environ.get("EDGECONV_TRACE", "0")))
    res = run_bass_kernel_spmd(nc, in_maps, core_ids=list(range(8)), trace=trace)
    if trace:
        kernel.last_exec_time_ns = res.exec_time_ns
    return assemble(cfg, res.results)


kernel.last_exec_time_ns = None
